# revision 7
# baseline (speedup 1.0000x reference)
"""Trainium2 Bass kernel for nn_AffinityMah (retrieval_knn).

Math (per batch b):
    out[n, m] = relu( ||Y[b,n] @ A||^2 + ||X[b,m] @ A||^2 - 2 * (YA @ XA^T)[n, m] )

Strategy:
  - Data-parallel over batch B=8 across the 8 NeuronCores (one batch per core).
  - Per core, the whole quadratic form is folded into ONE TensorE matmul per
    output tile by augmenting the contraction dim to K+2 = 102:
        lhsT rows 0..99  = YA^T            rhs rows 0..99  = -2 * XA^T
        lhsT row  100    = sqY             rhs row  100    = ones
        lhsT row  101    = ones            rhs row  101    = sqX
    so  out_tile = lhsT.T @ rhs = sqY[:,None] + sqX[None,:] - 2*cross  directly
    in PSUM; a relu copy (ACT/DVE alternating) moves it to SBUF, then 1 MB DMAs
    write each 128-row block of the (2048, 2048) output.
  - X^T / Y^T (needed because TensorE contracts over partitions) are produced
    with PE transposes of 128x128 tiles; XA^T/YA^T then come from matmuls
    against A chunks; sq row vectors from a ones-vector matmul over squared
    entries.  float32r (replicated fp32) is used for all non-transpose matmuls
    (1 cycle/row at free-dim >= 256 vs 4 for plain fp32).
"""

import numpy as np

B, MX, NY, D, K = 8, 2048, 2048, 256, 100
KP = K + 2  # augmented contraction dim
S = 512     # moving-operand slice width
NS = MX // S          # 4 column slices
JT = NY // 128        # 16 output row blocks

_NC = None


def _emit(tc, O, X, Y, A):
    from contextlib import ExitStack

    import concourse.mybir as mybir
    from concourse.masks import make_identity

    nc = tc.nc
    f32 = mybir.dt.float32
    f32r = mybir.dt.float32r
    AF = mybir.ActivationFunctionType

    with ExitStack() as ctx:
        const = ctx.enter_context(tc.tile_pool(name="const", bufs=1))
        lr = ctx.enter_context(tc.tile_pool(name="lr", bufs=1))
        xin = ctx.enter_context(tc.tile_pool(name="xin", bufs=3))
        xt = ctx.enter_context(tc.tile_pool(name="xt", bufs=2))
        sqp = ctx.enter_context(tc.tile_pool(name="sqp", bufs=2))
        obp = ctx.enter_context(tc.tile_pool(name="obp", bufs=3))
        pt = ctx.enter_context(tc.tile_pool(name="pt", bufs=2, space="PSUM"))
        pa = ctx.enter_context(tc.tile_pool(name="pa", bufs=2, space="PSUM"))
        ps = ctx.enter_context(tc.tile_pool(name="ps", bufs=1, space="PSUM"))
        po = ctx.enter_context(tc.tile_pool(name="po", bufs=3, space="PSUM"))

        ident = const.tile([128, 128], f32, name="ident")
        make_identity(nc, ident[:])

        a_chunks = []
        for c in range(2):
            araw = const.tile([128, K], f32, name=f"araw{c}", tag=f"araw{c}")
            nc.sync.dma_start(araw[:], A[c * 128:(c + 1) * 128, :])
            ac = const.tile([128, K], f32r, name=f"a{c}", tag=f"a{c}")
            nc.vector.tensor_copy(ac[:], araw[:])
            a_chunks.append(ac)

        ones32 = const.tile([K, 1], f32, name="ones32", tag="ones32")
        nc.vector.memset(ones32[:], 1.0)
        ones_w = const.tile([K, 1], f32r, name="ones_w", tag="ones_w")
        nc.vector.tensor_copy(ones_w[:], ones32[:])

        ones_row32 = const.tile([1, S], f32, name="ones_row32", tag="ones_row32")
        nc.vector.memset(ones_row32[:], 1.0)
        ones_row = const.tile([1, S], f32r, name="ones_row", tag="ones_row")
        nc.vector.tensor_copy(ones_row[:], ones_row32[:])

        # L parts: [YA^T; sqY; ones], R parts: [-2 XA^T; ones; sqX]
        # Compute-engine writes must start at a 32-aligned partition, so rows
        # 100/101 are staged in a [2, S] tile at partition 0 and DMA'd in.
        Lp, Rp = [], []
        for s in range(NS):
            lt = lr.tile([KP, S], f32r, name=f"L{s}", tag=f"L{s}")
            Lp.append(lt)
            rt = lr.tile([KP, S], f32r, name=f"R{s}", tag=f"R{s}")
            Rp.append(rt)

        # ---- Stage A: build L and R (per input tensor, per 512-col slice) ----
        for ti, T in ((0, X), (1, Y)):
            for s in range(NS):
                # load the 512-row slab as [128, 4, 256]: partition p holds
                # rows s*512 + u*128 + p
                slab = xin.tile([128, NS, D], f32, name=f"slab{ti}{s}", tag="slab")
                nc.sync.dma_start(
                    slab[:], T[s * S:(s + 1) * S, :].rearrange("(u p) d -> p u d", p=128)
                )
                xts = [
                    xt.tile([128, S], f32r, name=f"xt{ti}{s}{c}", tag=f"xt{c}")
                    for c in range(2)
                ]
                for u in range(S // 128):
                    for c in range(2):
                        ptile = pt.tile([128, 128], f32, name=f"pt{ti}{s}{u}{c}", tag="pt")
                        nc.tensor.transpose(
                            ptile[:], slab[:, u, c * 128:(c + 1) * 128], ident[:]
                        )
                        nc.vector.tensor_copy(xts[c][:, u * 128:(u + 1) * 128], ptile[:])

                # XA^T / YA^T slice: accumulate over the two D-chunks
                pxa = pa.tile([K, S], f32, name=f"pxa{ti}{s}", tag="pa")
                nc.tensor.matmul(
                    pxa[:], a_chunks[0][:], xts[0][:],
                    start=True, stop=False,
                )
                nc.tensor.matmul(
                    pxa[:], a_chunks[1][:], xts[1][:],
                    start=False, stop=True,
                )

                sqt = sqp.tile([K, S], f32r, name=f"sq{ti}{s}", tag="sq")
                nc.scalar.square(sqt[:], pxa[:])
                if ti == 0:
                    nc.scalar.mul(Rp[s][0:K, :], pxa[:], -2.0)
                else:
                    nc.scalar.copy(Lp[s][0:K, :], pxa[:])

                pss = ps.tile([1, S], f32, name=f"pss{ti}{s}", tag="ps")
                nc.tensor.matmul(
                    pss[:], ones_w[:], sqt[:],
                    start=True, stop=True,
                )
                # rows 100 (L: sqY / R: ones) and 101 (L: ones / R: sqX):
                # compute writes must start 32-aligned, so stage the sq row at
                # partition 0 and DMA rows into place individually.
                sqrow = sqp.tile([1, S], f32r, name=f"sqrow{ti}{s}", tag="sqrow")
                nc.vector.tensor_copy(sqrow[:], pss[:])
                if ti == 0:
                    nc.sync.dma_start(Rp[s][K:K + 1, :], ones_row[:])
                    nc.sync.dma_start(Rp[s][K + 1:K + 2, :], sqrow[:])
                else:
                    nc.sync.dma_start(Lp[s][K:K + 1, :], sqrow[:])
                    nc.sync.dma_start(Lp[s][K + 1:K + 2, :], ones_row[:])

        # ---- Main loop: one matmul per (128, 512) output tile ----
        for j in range(JT):
            ob = obp.tile([128, MX], f32, name=f"ob{j}", tag="ob")
            for t in range(NS):
                pot = po.tile([128, S], f32, name=f"po{j}{t}", tag="po")
                nc.tensor.matmul(
                    pot[:],
                    Lp[j // 4][:, (j % 4) * 128:(j % 4 + 1) * 128],
                    Rp[t][:],
                    start=True, stop=True,
                )
                if (j * NS + t) % 2 == 0:
                    nc.scalar.activation(ob[:, t * S:(t + 1) * S], pot[:], AF.Relu)
                else:
                    nc.vector.tensor_relu(ob[:, t * S:(t + 1) * S], pot[:])
            nc.sync.dma_start(O[j * 128:(j + 1) * 128, :], ob[:])


def _build_nc():
    import concourse.bass as bass  # noqa: F401
    import concourse.mybir as mybir
    import concourse.tile as tile
    from concourse import bacc

    f32 = mybir.dt.float32
    nc = bacc.Bacc(
        "TRN2", target_bir_lowering=False, debug=False, enable_asserts=False
    )
    Xd = nc.dram_tensor("X", [MX, D], f32, kind="ExternalInput").ap()
    Yd = nc.dram_tensor("Y", [NY, D], f32, kind="ExternalInput").ap()
    Ad = nc.dram_tensor("A", [D, K], f32, kind="ExternalInput").ap()
    Od = nc.dram_tensor("O", [NY, MX], f32, kind="ExternalOutput").ap()

    with tile.TileContext(nc) as tc:
        _emit(tc, Od, Xd, Yd, Ad)
    nc.compile()
    return nc


def get_nc():
    global _NC
    if _NC is None:
        _NC = _build_nc()
    return _NC


def kernel(X, Y, A, _trace=False):
    from concourse.bass_utils import run_bass_kernel_spmd

    nc = get_nc()
    X = np.ascontiguousarray(X, dtype=np.float32)
    Y = np.ascontiguousarray(Y, dtype=np.float32)
    A = np.ascontiguousarray(A, dtype=np.float32)
    in_maps = [{"X": X[b], "Y": Y[b], "A": A} for b in range(B)]
    res = run_bass_kernel_spmd(nc, in_maps, core_ids=list(range(B)), trace=_trace)
    out = np.stack([res.results[b]["O"] for b in range(B)], axis=0)
    if _trace:
        return out, res
    return out


# revision 12
# speedup vs baseline: 1.0823x; 1.0823x over previous
"""Trainium2 Bass kernel for nn_AffinityMah (retrieval_knn).

Math (per batch b):
    out[n, m] = relu( ||Y[b,n] @ A||^2 + ||X[b,m] @ A||^2 - 2 * (YA @ XA^T)[n, m] )

Strategy:
  - Data-parallel over batch B=8 across the 8 NeuronCores (one batch per core).
  - Inputs are cast to bf16 on the host (halves input HBM traffic; the PE runs
    bf16 matmuls at 1 cycle/row with fast weight load).
  - X^T / Y^T are produced with PE transposes of 128x128 bf16 tiles (the DMA
    crossbar transpose hangs on this runtime), then DVE copies PSUM -> SBUF.
  - XA^T / YA^T slices come from matmuls against A chunks (contract D=256 in
    two 128-chunks, accumulate in PSUM); row-sums of squares from a
    ones-vector matmul over Square(XA^T).
  - The whole quadratic form is then ONE TensorE matmul per (128, 512) output
    tile via an augmented contraction dim K+2 = 102:
        lhsT rows 0..99  = YA^T            rhs rows 0..99  = -2 * XA^T
        lhsT row  100    = sqY             rhs row  100    = ones
        lhsT row  101    = ones            rhs row  101    = sqX
    giving out_tile = sqY[:,None] + sqX[None,:] - 2*cross directly in PSUM.
    A relu copy (ACT/DVE alternating) moves each tile to SBUF and a 256 KB
    DMA writes it out immediately (wavefront order so output DMA starts
    as early as possible).
"""

import numpy as np

B, MX, NY, D, K = 8, 2048, 2048, 256, 100
KP = K + 2  # augmented contraction dim
S = 512     # moving-operand slice width
NS = MX // S          # 4 column slices
JT = NY // 128        # 16 output row blocks

_NC = None


def _emit(tc, O, X, Y, A):
    from contextlib import ExitStack

    import concourse.mybir as mybir
    from concourse.masks import make_identity

    nc = tc.nc
    f32 = mybir.dt.float32
    bf16 = mybir.dt.bfloat16
    AF = mybir.ActivationFunctionType

    with ExitStack() as ctx:
        const = ctx.enter_context(tc.tile_pool(name="const", bufs=1))
        lr = ctx.enter_context(tc.tile_pool(name="lr", bufs=1))
        xin = ctx.enter_context(tc.tile_pool(name="xin", bufs=3))
        xt = ctx.enter_context(tc.tile_pool(name="xt", bufs=2))
        sqp = ctx.enter_context(tc.tile_pool(name="sqp", bufs=2))
        obp = ctx.enter_context(tc.tile_pool(name="obp", bufs=6))
        pt = ctx.enter_context(tc.tile_pool(name="pt", bufs=2, space="PSUM"))
        pa = ctx.enter_context(tc.tile_pool(name="pa", bufs=2, space="PSUM"))
        ps = ctx.enter_context(tc.tile_pool(name="ps", bufs=1, space="PSUM"))
        po = ctx.enter_context(tc.tile_pool(name="po", bufs=3, space="PSUM"))

        ident = const.tile([128, 128], bf16, name="ident")
        make_identity(nc, ident[:])

        a_chunks = []
        for c in range(2):
            ac = const.tile([128, K], bf16, name=f"a{c}", tag=f"a{c}")
            nc.sync.dma_start(ac[:], A[c * 128:(c + 1) * 128, :])
            a_chunks.append(ac)

        ones_w = const.tile([K, 1], bf16, name="ones_w", tag="ones_w")
        nc.vector.memset(ones_w[:], 1.0)
        ones_row = const.tile([1, S], bf16, name="ones_row", tag="ones_row")
        nc.vector.memset(ones_row[:], 1.0)

        # L parts: [YA^T; sqY; ones], R parts: [-2 XA^T; ones; sqX]
        Lp, Rp = [], []
        for s in range(NS):
            lt = lr.tile([KP, S], bf16, name=f"L{s}", tag=f"L{s}")
            Lp.append(lt)
            rt = lr.tile([KP, S], bf16, name=f"R{s}", tag=f"R{s}")
            Rp.append(rt)

        # ---- Stage A: build L and R (Y first so L slices are ready early) ----
        for s in range(NS):
            for ti, T in ((1, Y), (0, X)):
                # load the 512-row slab as [128, 4, 256]: partition p holds
                # rows s*512 + u*128 + p
                slab = xin.tile([128, NS, D], bf16, name=f"slab{ti}{s}", tag="slab")
                nc.sync.dma_start(
                    slab[:],
                    T[s * S:(s + 1) * S, :].rearrange("(u p) d -> p u d", p=128),
                )
                xts = [
                    xt.tile([128, S], bf16, name=f"xt{ti}{s}{c}", tag=f"xt{c}")
                    for c in range(2)
                ]
                for u in range(S // 128):
                    for c in range(2):
                        ptile = pt.tile([128, 128], bf16,
                                        name=f"pt{ti}{s}{u}{c}", tag="pt")
                        nc.tensor.transpose(
                            ptile[:], slab[:, u, c * 128:(c + 1) * 128], ident[:]
                        )
                        nc.vector.tensor_copy(
                            xts[c][:, u * 128:(u + 1) * 128], ptile[:]
                        )

                # XA^T / YA^T slice: accumulate over the two D-chunks
                pxa = pa.tile([K, S], f32, name=f"pxa{ti}{s}", tag="pa")
                nc.tensor.matmul(pxa[:], a_chunks[0][:], xts[0][:],
                                 start=True, stop=False)
                nc.tensor.matmul(pxa[:], a_chunks[1][:], xts[1][:],
                                 start=False, stop=True)

                sqt = sqp.tile([K, S], bf16, name=f"sq{ti}{s}", tag="sq")
                nc.scalar.square(sqt[:], pxa[:])
                if ti == 0:
                    nc.scalar.mul(Rp[s][0:K, :], pxa[:], -2.0)
                else:
                    nc.scalar.copy(Lp[s][0:K, :], pxa[:])

                pss = ps.tile([1, S], f32, name=f"pss{ti}{s}", tag="ps")
                nc.tensor.matmul(pss[:], ones_w[:], sqt[:], start=True, stop=True)

                # rows 100 (L: sqY / R: ones) and 101 (L: ones / R: sqX):
                # compute writes must start 32-aligned, so stage the sq row at
                # partition 0 and DMA rows into place individually.
                sqrow = sqp.tile([1, S], bf16, name=f"sqrow{ti}{s}", tag="sqrow")
                nc.vector.tensor_copy(sqrow[:], pss[:])
                if ti == 0:
                    nc.sync.dma_start(Rp[s][K:K + 1, :], ones_row[:])
                    nc.sync.dma_start(Rp[s][K + 1:K + 2, :], sqrow[:])
                else:
                    nc.sync.dma_start(Lp[s][K:K + 1, :], sqrow[:])
                    nc.sync.dma_start(Lp[s][K + 1:K + 2, :], ones_row[:])

        # ---- Main loop: one matmul per (128, 512) output tile, wavefront ----
        tiles = [(j, t) for j in range(JT) for t in range(NS)]
        tiles.sort(key=lambda jt: (max(jt[0] // 4, jt[1]), jt[1], jt[0]))
        for i, (j, t) in enumerate(tiles):
            pot = po.tile([128, S], f32, name=f"po{j}_{t}", tag="po")
            nc.tensor.matmul(
                pot[:],
                Lp[j // 4][:, (j % 4) * 128:(j % 4 + 1) * 128],
                Rp[t][:],
                start=True, stop=True,
            )
            ot = obp.tile([128, S], f32, name=f"ot{j}_{t}", tag="ot")
            if i % 2 == 0:
                nc.scalar.activation(ot[:], pot[:], AF.Relu)
            else:
                nc.vector.tensor_relu(ot[:], pot[:])
            nc.sync.dma_start(O[j * 128:(j + 1) * 128, t * S:(t + 1) * S], ot[:])


def _build_nc():
    import concourse.bass as bass  # noqa: F401
    import concourse.mybir as mybir
    import concourse.tile as tile
    from concourse import bacc

    f32 = mybir.dt.float32
    bf16 = mybir.dt.bfloat16
    nc = bacc.Bacc(
        "TRN2", target_bir_lowering=False, debug=False, enable_asserts=False
    )
    Xd = nc.dram_tensor("X", [MX, D], bf16, kind="ExternalInput").ap()
    Yd = nc.dram_tensor("Y", [NY, D], bf16, kind="ExternalInput").ap()
    Ad = nc.dram_tensor("A", [D, K], bf16, kind="ExternalInput").ap()
    Od = nc.dram_tensor("O", [NY, MX], f32, kind="ExternalOutput").ap()

    with tile.TileContext(nc) as tc:
        _emit(tc, Od, Xd, Yd, Ad)
    nc.compile()
    return nc


def get_nc():
    global _NC
    if _NC is None:
        _NC = _build_nc()
    return _NC


def kernel(X, Y, A, _trace=False):
    import ml_dtypes

    from concourse.bass_utils import run_bass_kernel_spmd

    nc = get_nc()
    bf16 = ml_dtypes.bfloat16
    Xb = np.ascontiguousarray(X, dtype=np.float32).astype(bf16)
    Yb = np.ascontiguousarray(Y, dtype=np.float32).astype(bf16)
    Ab = np.ascontiguousarray(A, dtype=np.float32).astype(bf16)
    in_maps = [{"X": Xb[b], "Y": Yb[b], "A": Ab} for b in range(B)]
    res = run_bass_kernel_spmd(nc, in_maps, core_ids=list(range(B)), trace=_trace)
    out = np.stack([res.results[b]["O"] for b in range(B)], axis=0)
    if _trace:
        return out, res
    return out
